# revision 4
# baseline (speedup 1.0000x reference)
import sys

sys.path.insert(0, "/opt/trn_rl_repo")
import numpy as np
import concourse.bass as bass
import concourse.mybir as mybir
from concourse.tile import TileContext
from concourse.vector_clock import ScopedClock
from concourse.bass_utils import run_bass_kernel_spmd

NCORES = 8
P = 128
FP = mybir.dt.float32
S = 9

N_ATOM, N_RES = 32768, 4096
F_ATOM, F_RES = 64, 128
H_EDGE, RES_H = 64, 128
E_BOND, E_RAD = 65536, 393216
E_ATOM = E_BOND + E_RAD
E_RES = 131072


class SplitDrainTileContext(TileContext):
    # Walrus in this env rejects instructions with >1 sem wait ("Too many
    # sync wait commands"); split the exit-drain's waits into single-wait
    # NOPs on the same (in-order) SP queue — semantics unchanged.
    def _drain_and_barrier(self, tick_clock, wait_clock):
        nc = self.nc
        drain_inst = nc.sync.drain()
        wait_clock.add_sem_waits(
            drain_inst.ins, ScopedClock({None: tick_clock.global_clock})
        )
        waits = list(drain_inst.ins.sync_info.on_wait or [])
        if len(waits) > 1:
            drain_inst.ins.sync_info.on_wait = [waits[0]]
            for w in waits[1:]:
                nop = nc.sync.nop(nofuse=True)
                if nop.ins.sync_info is None:
                    nop.ins.sync_info = mybir.SyncInfo(on_wait=[w], on_update=[])
                else:
                    nop.ins.sync_info.on_wait = [w]
        nc.all_engine_barrier()
        assert self.sems is not None
        popped = nc._tile_sem_poison_stack.pop()
        assert popped is self._sem_poison
        nc.clear_and_free_semaphores(list(self.sems.allocated().values()))
        nc.all_engine_barrier()


def _cdiv(a, b):
    return -(-a // b)


def _split_multi_waits(nc):
    # Walrus in this env rejects instructions carrying >1 sem wait. Insert
    # single-wait NoOps on the same engine immediately before such
    # instructions — the sequencer resolves waits in order before executing,
    # so semantics are unchanged. Nops are created via the engine API (so
    # they are properly registered) and then moved into position.
    ET = mybir.EngineType
    eng = {
        ET.PE: nc.tensor,
        ET.DVE: nc.vector,
        ET.Activation: nc.scalar,
        ET.Pool: nc.gpsimd,
        ET.SP: nc.sync,
    }
    edits = []  # (bb, inst_name -> [nop insts])
    created = set()
    for f in nc.m.functions:
        for bb in f.blocks:
            pre = {}
            for inst in list(bb.instructions):
                si = inst.sync_info
                waits = list(si.on_wait) if (si is not None and si.on_wait) else []
                if len(waits) > 1:
                    nops = []
                    for w in waits[:-1]:
                        nop = eng[inst.engine].nop(nofuse=True).ins
                        if nop.sync_info is None:
                            nop.sync_info = mybir.SyncInfo(on_wait=[w], on_update=[])
                        else:
                            nop.sync_info.on_wait = [w]
                        nops.append(nop)
                        created.add(nop.name)
                    si.on_wait = [waits[-1]]
                    pre[inst.name] = nops
            if pre:
                edits.append((bb, pre))
    if not created:
        return
    for f in nc.m.functions:
        for bb in f.blocks:
            insts = bb.instructions
            if any(i.name in created for i in insts):
                insts[:] = [i for i in insts if i.name not in created]
    for bb, pre in edits:
        insts = bb.instructions
        out = []
        for inst in insts:
            out.extend(pre.get(inst.name, ()))
            out.append(inst)
        insts[:] = out


def _mid_bcast(ap2d, count):
    # [P, k] AP -> [P, count, k] with the middle dim broadcast (step 0)
    return bass.AP(
        tensor=ap2d.tensor,
        offset=ap2d.offset,
        ap=[ap2d.ap[0], [0, count], ap2d.ap[1]],
    )


def _view3(ap2d, F, s=S):
    # [P, F*s] AP (contiguous) -> [P, F, s]
    return bass.AP(
        tensor=ap2d.tensor,
        offset=ap2d.offset,
        ap=[ap2d.ap[0], [s, F], [1, s]],
    )


def _prep_conv(n_nodes, groups, resid, node_cnt):
    """groups: list of dicts(dst[int64 E], ef [E,H], x [E,F_in], sh [E,9]).
    Returns (caps, per-core input arrays dict list, layout info)."""
    nb = n_nodes // P // NCORES  # block slots per core
    NBG = n_nodes // P
    F_in = groups[0]["x"].shape[1]
    H = groups[0]["ef"].shape[1]
    ng = len(groups)

    caps = []  # caps[g] = [nb] per-slot tile capacity (same for all cores)
    sorted_groups = []
    for gd in groups:
        order = np.argsort(gd["dst"], kind="stable")
        dst_s = np.asarray(gd["dst"])[order]
        gb = dst_s // P
        counts = np.bincount(gb, minlength=NBG).astype(np.int64)
        cnt_cb = counts.reshape(NCORES, nb)
        caps_b = _cdiv(cnt_cb, P).max(axis=0)  # [nb]
        caps.append(caps_b)
        run_start = np.concatenate([[0], np.cumsum(counts)[:-1]])
        rank = np.arange(len(dst_s)) - run_start[gb]
        sorted_groups.append((order, dst_s, gb, counts, rank))

    tiles_per_block = np.sum([c for c in caps], axis=0)  # [nb]
    block_tile_base = np.concatenate([[0], np.cumsum(tiles_per_block)[:-1]])
    NT = int(tiles_per_block.sum())
    Ecap = NT * P

    # per-(slot, group) tile base
    gt_base = np.zeros((nb, ng), dtype=np.int64)
    for b in range(nb):
        off = block_tile_base[b]
        for g in range(ng):
            gt_base[b, g] = off
            off += caps[g][b]

    per_core = []
    Nsh = n_nodes // NCORES
    for c in range(NCORES):
        ef_pad = np.zeros((Ecap, H), np.float32)
        x_pad = np.zeros((Ecap, F_in), np.float32)
        sh_pad = np.zeros((Ecap, S), np.float32)
        dst_pad = np.full((Ecap,), -1.0, np.float32)
        for g, (order, dst_s, gb, counts, rank) in enumerate(sorted_groups):
            sel = (gb >= c * nb) & (gb < (c + 1) * nb)
            bl = gb[sel] - c * nb  # local slot
            pos = gt_base[bl, g] * P + rank[sel]
            oidx = order[sel]
            gd = groups[g]
            ef_pad[pos] = np.asarray(gd["ef"], np.float32)[oidx]
            x_pad[pos] = np.asarray(gd["x"], np.float32)[oidx]
            sh_pad[pos] = np.asarray(gd["sh"], np.float32)[oidx]
            dst_pad[pos] = (dst_s[sel] - (gb[sel] * P)).astype(np.float32)

        arrs = {
            "efT": np.ascontiguousarray(ef_pad.T),
            "xpk": np.ascontiguousarray(
                x_pad.reshape(NT, P, F_in).transpose(1, 0, 2).reshape(P, NT * F_in)
            ),
            "shpk": np.ascontiguousarray(
                sh_pad.reshape(NT, P, S).transpose(1, 0, 2).reshape(P, NT * S)
            ),
            "dstT": np.ascontiguousarray(dst_pad.reshape(NT, P).T),
            "invc": np.ascontiguousarray(
                (
                    1.0
                    / np.maximum(node_cnt[c * Nsh : (c + 1) * Nsh], 1.0)
                ).reshape(nb, P).T.astype(np.float32)
            ),
            "residT": np.ascontiguousarray(
                np.asarray(resid, np.float32)[c * Nsh : (c + 1) * Nsh].T
            ),
        }
        per_core.append(arrs)
    return caps, per_core, dict(nb=nb, NT=NT, Ecap=Ecap, Nsh=Nsh, H=H, F_in=F_in)


def _build_conv(H, F_in, F_out, caps, nb, NT, Nsh, wn_bufs):
    WN = F_in * S
    Ecap = NT * P
    ng = len(caps)
    nch = _cdiv(WN, P)
    n512 = [(i * 512, min((i + 1) * 512, WN)) for i in range(_cdiv(WN, 512))]
    CH = 16  # tiles per DMA chunk

    nc = bass.Bass()
    d_efT = nc.declare_dram_parameter("efT", [H, Ecap], FP, isOutput=False)
    d_xpk = nc.declare_dram_parameter("xpk", [P, NT * F_in], FP, isOutput=False)
    d_shpk = nc.declare_dram_parameter("shpk", [P, NT * S], FP, isOutput=False)
    d_dstT = nc.declare_dram_parameter("dstT", [P, NT], FP, isOutput=False)
    d_invc = nc.declare_dram_parameter("invc", [P, nb], FP, isOutput=False)
    d_residT = nc.declare_dram_parameter("residT", [F_out, Nsh], FP, isOutput=False)
    d_W1, d_b1, d_W2, d_b2 = [], [], [], []
    for g in range(ng):
        d_W1.append(nc.declare_dram_parameter(f"W1_{g}", [H, H], FP, isOutput=False))
        d_b1.append(nc.declare_dram_parameter(f"b1_{g}", [H, 1], FP, isOutput=False))
        d_W2.append(nc.declare_dram_parameter(f"W2_{g}", [H, WN], FP, isOutput=False))
        d_b2.append(nc.declare_dram_parameter(f"b2_{g}", [1, WN], FP, isOutput=False))
    d_WoutPk = nc.declare_dram_parameter("WoutPk", [P, nch * F_out], FP, isOutput=False)
    d_iota = nc.declare_dram_parameter("iota", [P, P], FP, isOutput=False)
    d_ident = nc.declare_dram_parameter("ident", [P, P], FP, isOutput=False)
    d_ones = nc.declare_dram_parameter("ones", [1, P], FP, isOutput=False)
    d_outT = nc.declare_dram_parameter("outT", [F_out, Nsh], FP, isOutput=True)

    AL = mybir.AluOpType
    AF = mybir.ActivationFunctionType

    with SplitDrainTileContext(nc) as tc:
        with (
            tc.tile_pool(name="const", bufs=1) as cst,
            tc.tile_pool(name="stream", bufs=2) as stm,
            tc.tile_pool(name="work", bufs=2) as wrk,
            tc.tile_pool(name="pss", bufs=2, space="PSUM") as pss,
            tc.tile_pool(name="psw", bufs=wn_bufs, space="PSUM") as psw,
            tc.tile_pool(name="psm", bufs=1, space="PSUM") as psm,
        ):
            # ---- preload constants ----
            sW1, sb1, sW2, sb2 = [], [], [], []
            for g in range(ng):
                t = cst.tile([H, H], FP, tag=f"W1_{g}")
                nc.sync.dma_start(out=t[:, :], in_=d_W1[g][:, :])
                sW1.append(t)
                t = cst.tile([H, 1], FP, tag=f"b1_{g}")
                nc.sync.dma_start(out=t[:, :], in_=d_b1[g][:, :])
                sb1.append(t)
                t = cst.tile([H, WN], FP, tag=f"W2_{g}")
                nc.sync.dma_start(out=t[:, :], in_=d_W2[g][:, :])
                sW2.append(t)
                t = cst.tile([1, WN], FP, tag=f"b2_{g}")
                nc.sync.dma_start(out=t[:, :], in_=d_b2[g][:, :])
                sb2.append(t)
            sWout = cst.tile([P, nch * F_out], FP, tag="Wout")
            nc.sync.dma_start(out=sWout[:, :], in_=d_WoutPk[:, :])
            siota = cst.tile([P, P], FP, tag="iota")
            nc.sync.dma_start(out=siota[:, :], in_=d_iota[:, :])
            sident = cst.tile([P, P], FP, tag="ident")
            nc.sync.dma_start(out=sident[:, :], in_=d_ident[:, :])
            sones = cst.tile([1, P], FP, tag="ones")
            nc.sync.dma_start(out=sones[:, :], in_=d_ones[:, :])
            sdstT = cst.tile([P, NT], FP, tag="dstT")
            nc.sync.dma_start(out=sdstT[:, :], in_=d_dstT[:, :])
            sinvc = cst.tile([P, nb], FP, tag="invc")
            nc.sync.dma_start(out=sinvc[:, :], in_=d_invc[:, :])
            sresid = cst.tile([F_out, Nsh], FP, tag="resid")
            nc.sync.dma_start(out=sresid[:, :], in_=d_residT[:, :])

            gt = 0
            efc = xc = shc = None
            for b in range(nb):
                tiles_in_block = int(sum(caps[g][b] for g in range(ng)))
                outtile = wrk.tile([F_out, P], FP, tag="outtile")
                if tiles_in_block == 0:
                    nc.vector.tensor_copy(
                        out=outtile[:, :], in_=sresid[:, b * P : (b + 1) * P]
                    )
                    nc.sync.dma_start(
                        out=d_outT[:, b * P : (b + 1) * P], in_=outtile[:, :]
                    )
                    continue
                pM1 = psm.tile([P, WN], FP, tag="pM1")
                ti = 0
                for g in range(ng):
                    for _t in range(int(caps[g][b])):
                        if gt % CH == 0:
                            cur = min(CH, NT - gt)
                            efc = stm.tile([H, CH * P], FP, tag="efc")
                            nc.sync.dma_start(
                                out=efc[:, : cur * P],
                                in_=d_efT[:, gt * P : (gt + cur) * P],
                            )
                            xc = stm.tile([P, CH * F_in], FP, tag="xc")
                            nc.sync.dma_start(
                                out=xc[:, : cur * F_in],
                                in_=d_xpk[:, gt * F_in : (gt + cur) * F_in],
                            )
                            shc = stm.tile([P, CH * S], FP, tag="shc")
                            nc.sync.dma_start(
                                out=shc[:, : cur * S],
                                in_=d_shpk[:, gt * S : (gt + cur) * S],
                            )
                        i = gt % CH
                        # mm1: h[hid, e] = W1.T @ efT-tile
                        ph = pss.tile([H, P], FP, tag="pss")
                        nc.tensor.matmul(
                            ph[:, :],
                            sW1[g][:, :],
                            efc[:, i * P : (i + 1) * P],
                            start=True,
                            stop=True,
                        )
                        # relu(h + b1) -> SBUF [H, e]
                        hS = wrk.tile([H, P], FP, tag="hS")
                        nc.scalar.activation(
                            hS[:, :], ph[:, :], AF.Relu, bias=sb1[g][:, :]
                        )
                        # mm2: w[e, WN] = hS.T @ W2 (+ ones.T @ b2)
                        pw = psw.tile([P, WN], FP, tag="pw")
                        for (a0, a1) in n512:
                            nc.tensor.matmul(
                                pw[:, a0:a1],
                                hS[:, :],
                                sW2[g][:, a0:a1],
                                start=True,
                                stop=False,
                                skip_group_check=True,
                            )
                            nc.tensor.matmul(
                                pw[:, a0:a1],
                                sones[:, :],
                                sb2[g][:, a0:a1],
                                start=False,
                                stop=True,
                                skip_group_check=True,
                            )
                        # Hadamard: v = w * x_bcast * sh_bcast  (f-major: (f,s))
                        xs = xc[:, i * F_in : (i + 1) * F_in]
                        shs = shc[:, i * S : (i + 1) * S]
                        vt = wrk.tile([P, WN], FP, tag="vt")
                        nc.vector.tensor_tensor(
                            out=_view3(vt[:, :], F_in),
                            in0=_view3(pw[:, :], F_in),
                            in1=xs.to_broadcast([P, F_in, S]),
                            op=AL.mult,
                        )
                        v = wrk.tile([P, WN], FP, tag="v")
                        nc.vector.tensor_tensor(
                            out=_view3(v[:, :], F_in),
                            in0=_view3(vt[:, :], F_in),
                            in1=_mid_bcast(shs, F_in),
                            op=AL.mult,
                        )
                        # one-hot scatter matrix from dst indices
                        St = wrk.tile([P, P], FP, tag="St")
                        nc.vector.tensor_scalar(
                            St[:, :],
                            siota[:, :],
                            sdstT[:, gt : gt + 1],
                            None,
                            AL.is_equal,
                        )
                        # scatter-accumulate M1[d, WN] += S.T @ v
                        for (a0, a1) in n512:
                            nc.tensor.matmul(
                                pM1[:, a0:a1],
                                St[:, :],
                                v[:, a0:a1],
                                start=(ti == 0),
                                stop=(ti == tiles_in_block - 1),
                                skip_group_check=True,
                            )
                        ti += 1
                        gt += 1
                # ---- block reduce: mean scale, transpose, Wout, residual ----
                M1S = wrk.tile([P, WN], FP, tag="M1S")
                nc.vector.tensor_scalar(
                    M1S[:, :], pM1[:, :], sinvc[:, b : b + 1], None, AL.mult
                )
                pout = pss.tile([F_out, P], FP, tag="pss")
                for j in range(nch):
                    cw = min(P, WN - j * P)
                    pT = pss.tile([P, P], FP, tag="pss")
                    nc.tensor.transpose(
                        pT[0:cw, :], M1S[:, j * P : j * P + cw], sident[:, :]
                    )
                    M1T = wrk.tile([P, P], FP, tag="M1T")
                    nc.vector.tensor_copy(out=M1T[0:cw, :], in_=pT[0:cw, :])
                    nc.tensor.matmul(
                        pout[:, :],
                        sWout[0:cw, j * F_out : (j + 1) * F_out],
                        M1T[0:cw, :],
                        start=(j == 0),
                        stop=(j == nch - 1),
                        skip_group_check=True,
                    )
                nc.vector.tensor_tensor(
                    out=outtile[:, :],
                    in0=pout[:, :],
                    in1=sresid[:, b * P : (b + 1) * P],
                    op=AL.add,
                )
                nc.sync.dma_start(
                    out=d_outT[:, b * P : (b + 1) * P], in_=outtile[:, :]
                )
    return nc


def _pack_wout(Wout, F_out):
    WN = Wout.shape[0]
    nch = _cdiv(WN, P)
    pk = np.zeros((P, nch * F_out), np.float32)
    for j in range(nch):
        cw = min(P, WN - j * P)
        pk[:cw, j * F_out : (j + 1) * F_out] = Wout[j * P : j * P + cw]
    return pk


def _run_conv(n_nodes, F_out, groups, weights, Wout, resid, wn_bufs=2):
    """groups: list of dicts(dst, ef, x, sh); weights: list of (W1,b1,W2,b2)."""
    dst_all = np.concatenate([g["dst"] for g in groups])
    node_cnt = np.bincount(dst_all, minlength=n_nodes).astype(np.float32)
    caps, per_core, info = _prep_conv(n_nodes, groups, resid, node_cnt)
    H, F_in, nb, NT, Nsh = info["H"], info["F_in"], info["nb"], info["NT"], info["Nsh"]
    nc = _build_conv(H, F_in, F_out, caps, nb, NT, Nsh, wn_bufs)

    shared = {
        "WoutPk": _pack_wout(np.asarray(Wout, np.float32), F_out),
        "iota": np.tile(np.arange(P, dtype=np.float32), (P, 1)),
        "ident": np.eye(P, dtype=np.float32),
        "ones": np.ones((1, P), np.float32),
    }
    for g, (W1, b1, W2, b2) in enumerate(weights):
        shared[f"W1_{g}"] = np.ascontiguousarray(np.asarray(W1, np.float32))
        shared[f"b1_{g}"] = np.asarray(b1, np.float32).reshape(H, 1)
        shared[f"W2_{g}"] = np.ascontiguousarray(np.asarray(W2, np.float32))
        shared[f"b2_{g}"] = np.asarray(b2, np.float32).reshape(1, -1)
    in_maps = [{**per_core[c], **shared} for c in range(NCORES)]
    _split_multi_waits(nc)
    res = run_bass_kernel_spmd(nc, in_maps, list(range(NCORES)))
    out = np.concatenate([res.results[c]["outT"].T for c in range(NCORES)], axis=0)
    return out


def kernel(
    atom_features, res_features, atom_edge_index, bond_features,
    radius_edge_features, atom_edge_sh, atom_res_batch,
    agg_edge_features, agg_edge_sh, res_edge_index,
    res_edge_features, res_edge_sh,
    Wb1, bb1, Wb2, bb2, Wr1, br1, Wr2, br2, W_atom_out,
    Wa1, ba1, Wa2, ba2, W_agg_out,
    Ws1, bs1, Ws2, bs2, W_res_out,
):
    af = np.asarray(atom_features, np.float32)
    rf = np.asarray(res_features, np.float32)
    aei = np.asarray(atom_edge_index)
    src_a, dst_a = aei[0], aei[1]
    sh_a = np.asarray(atom_edge_sh, np.float32)
    arb = np.asarray(atom_res_batch)

    # ---- conv1: atom graph (bond + radius groups) ----
    groups1 = [
        dict(
            dst=np.asarray(dst_a[:E_BOND]),
            ef=np.asarray(bond_features, np.float32),
            x=af[np.asarray(src_a[:E_BOND])],
            sh=sh_a[:E_BOND],
        ),
        dict(
            dst=np.asarray(dst_a[E_BOND:]),
            ef=np.asarray(radius_edge_features, np.float32),
            x=af[np.asarray(src_a[E_BOND:])],
            sh=sh_a[E_BOND:],
        ),
    ]
    atom_out = _run_conv(
        N_ATOM, F_ATOM, groups1,
        [(Wb1, bb1, Wb2, bb2), (Wr1, br1, Wr2, br2)],
        W_atom_out, af, wn_bufs=2,
    )

    # ---- conv2: atom -> residue aggregation (one edge per atom) ----
    groups2 = [
        dict(
            dst=arb,
            ef=np.asarray(agg_edge_features, np.float32),
            x=atom_out,
            sh=np.asarray(agg_edge_sh, np.float32),
        )
    ]
    res_mid = _run_conv(
        N_RES, F_RES, groups2, [(Wa1, ba1, Wa2, ba2)], W_agg_out, rf, wn_bufs=2
    )

    # ---- conv3: residue graph ----
    rei = np.asarray(res_edge_index)
    groups3 = [
        dict(
            dst=np.asarray(rei[1]),
            ef=np.asarray(res_edge_features, np.float32),
            x=res_mid[np.asarray(rei[0])],
            sh=np.asarray(res_edge_sh, np.float32),
        )
    ]
    res_out = _run_conv(
        N_RES, F_RES, groups3, [(Ws1, bs1, Ws2, bs2)], W_res_out, res_mid, wn_bufs=1
    )

    return atom_out, res_out
